# revision 31
# baseline (speedup 1.0000x reference)
"""CopyGenerator kernel for 8 Trainium2 NeuronCores.

Strategy: tensor-parallel over the vocab dimension, collective-free.
  - Each core computes logits = hidden @ W[:, k*4000:(k+1)*4000] (bf16 matmul,
    fp32 accumulate) and applies exp via ACT with a per-row bias ln(1-p_copy),
    so the activation directly emits e = exp(logit)*(1-p_copy) in bf16,
    streamed to DRAM, with the fused row-sum (accum_out) kept as fp32
    partials.
  - No AllReduce: the softmax denominator is finished on the host - each
    core returns its [128, 2, 16] row-sum partials (16 KB) and the host
    sums them across cores and applies the 1/Z row scale while upcasting
    the bf16 shards to the fp32 output.
  - p_copy = sigmoid(hidden @ Wc + bc) is a [2048,512]x[512,1] matvec,
    computed on the host; the device receives ln(1-p_copy) as an ACT bias
    and a pre-scaled attention (attn * p_copy) for the copy path.
  - Copy path (einsum over src_map) sharded 4 batches per core on the PE,
    emitted before the main loop so it runs while W streams in.
Host side: shard/cast inputs, run SPMD on cores 0-7, normalize + gather.
"""

import numpy as np
import ml_dtypes

bf16 = ml_dtypes.bfloat16

# Problem shape (hardcoded per contract)
B, T, S, C, D, V = 32, 64, 400, 100, 512, 32000
R = B * T              # 2048 rows, row r = t*32 + b
NC = 8
VS = V // NC           # 4000 vocab cols per core
PAD_IDX = 1
NEG_INF = -1e9

KCH = D // 128         # 4 contraction chunks of 128
NRB = R // 128         # 16 row blocks
SCH = 4                # s-chunks of 100 for the copy einsum
OUT_BUFS = 4

_cache = {}


def _build(all_bias: bool):
    import concourse.bass as bass
    import concourse.mybir as mybir
    import concourse.tile as tile
    from concourse import bacc

    fp32 = mybir.dt.float32
    bf = mybir.dt.bfloat16
    AF = mybir.ActivationFunctionType

    nc = bacc.Bacc("TRN2", target_bir_lowering=False, debug=False, num_devices=NC)

    # ---- I/O ----
    hT_d = nc.dram_tensor("hT", [D, R], bf, kind="ExternalInput")
    W_d = nc.dram_tensor("Wk", [D, VS], bf, kind="ExternalInput")
    lnb_d = nc.dram_tensor("lnb", [128, NRB], fp32, kind="ExternalInput")
    attnT_d = nc.dram_tensor("attnT", [S, 256], bf, kind="ExternalInput")
    srcmap_d = nc.dram_tensor("srcmap", [S, 4 * C], bf, kind="ExternalInput")
    out_d = nc.dram_tensor("out", [R, VS], bf, kind="ExternalOutput")
    rs_d = nc.dram_tensor("rs", [128, 8 * NRB], fp32, kind="ExternalOutput")
    cp_d = nc.dram_tensor("cp", [T, 4 * C], fp32, kind="ExternalOutput")
    if all_bias:
        bias_d = nc.dram_tensor("biask", [1, VS], bf, kind="ExternalInput")

    with tile.TileContext(nc) as tc:
        with (
            tc.tile_pool(name="sb", bufs=1) as sb,
            tc.tile_pool(name="ps", bufs=4, space="PSUM") as ps,
        ):
            # ---- resident loads ----
            # sync (HWDGE) ring leads with the eight 0.5MB W column-chunks
            # that gate the matmul stream; the bulk of hT rides at its tail.
            # The gpsimd SWDGE ring carries, in parallel: lnb, the first hT
            # row-chunks (which gate the first stripes), and the copy-path
            # inputs.
            hT_sb = sb.tile([128, KCH, R], bf)
            hT_view = hT_d.ap().rearrange("(c p) r -> p c r", p=128)
            W_sb = sb.tile([128, KCH, VS], bf)
            W_view = W_d.ap().rearrange("(c p) v -> p c v", p=128)
            # sync ring: hT head + all W chunks; gpsimd ring (parallel):
            # lnb, hT row-chunk 1, copy-path inputs, hT bulk.
            nc.sync.dma_start(hT_sb[:, :, 0:384], hT_view[:, :, 0:384])
            for q in range(8):
                nc.sync.dma_start(W_sb[:, :, q * 500:(q + 1) * 500],
                                  W_view[:, :, q * 500:(q + 1) * 500])

            lnb_sb = sb.tile([128, NRB], fp32)
            nc.gpsimd.dma_start(lnb_sb[:, :], lnb_d.ap())
            nc.gpsimd.dma_start(hT_sb[:, :, 384:512], hT_view[:, :, 384:512])
            attnT_sb = sb.tile([100, SCH, 256], bf)
            nc.gpsimd.dma_start(attnT_sb[:, :, :], attnT_d.ap().rearrange("(c p) j -> p c j", p=100))
            srcmap_sb = sb.tile([100, SCH, 4 * C], bf)
            nc.gpsimd.dma_start(srcmap_sb[:, :, :], srcmap_d.ap().rearrange("(c p) j -> p c j", p=100))
            for rq in range(1, 4):
                nc.gpsimd.dma_start(hT_sb[:, :, rq * 512:(rq + 1) * 512],
                                    hT_view[:, :, rq * 512:(rq + 1) * 512])
            if all_bias:
                bias_sb = sb.tile([1, VS], bf)
                nc.gpsimd.dma_start(bias_sb[:, :], bias_d.ap())
                ones_sb = sb.tile([1, 128], bf)
                nc.vector.memset(ones_sb[:, :], 1.0)

            rs_sb = sb.tile([128, 8 * NRB], fp32)  # rowsum partials [p, rb*8+c]
            nc.vector.memset(rs_sb[:, :], 0.0)
            cp_sb = sb.tile([64, 4 * C], fp32)

            ot_tiles = {}

            def get_ot(rb):
                if rb not in ot_tiles:
                    ot_tiles[rb] = sb.tile([128, VS], bf, tag="ot",
                                           bufs=OUT_BUFS, name=f"ot{rb}")
                return ot_tiles[rb]

            def stripe(rb, c0, nb, accum=True):
                """One nb*500-col stripe: matmuls + exp with fused bias/accum."""
                ot = get_ot(rb)
                st = ps.tile([128, 2, 512], fp32, tag="stripe",
                             name=f"l{rb}_{c0}")
                for kk in range(KCH):
                    for j in range(nb):
                        nc.tensor.matmul(
                            st[:, j, 0:500],
                            hT_sb[:, kk, rb * 128:(rb + 1) * 128],
                            W_sb[:, kk, (c0 + j) * 500:(c0 + j + 1) * 500],
                            start=(kk == 0),
                            stop=(kk == KCH - 1 and not all_bias))
                if all_bias:
                    for j in range(nb):
                        nc.tensor.matmul(
                            st[:, j, 0:500], ones_sb[:, :],
                            bias_sb[:, (c0 + j) * 500:(c0 + j + 1) * 500],
                            start=False, stop=True)
                ev = ot[:, c0 * 500:(c0 + nb) * 500]
                if nb > 1:
                    ev = ev.rearrange("p (g v) -> p g v", g=nb)
                    si = st[:, :, 0:500]
                else:
                    si = st[:, 0, 0:500]
                nc.scalar.activation(
                    ev, si, AF.Exp,
                    bias=lnb_sb[:, rb:rb + 1],
                    accum_out=(rs_sb[:, rb * 8 + c0:rb * 8 + c0 + 1]
                               if accum else None))

            def emit_out(rb, c0=0, c1=8):
                nc.sync.dma_start(
                    out_d.ap()[rb * 128:(rb + 1) * 128, c0 * 500:c1 * 500],
                    ot_tiles[rb][:, c0 * 500:c1 * 500])

            # ---- PE warmup: ~3.4us of dummy matmuls on zero tiles so the
            # HAM clock gate is at 8/8 (2.4 GHz) when the real stream
            # starts; they also fill the input-DMA wait ----
            wu_w = sb.tile([128, 128], bf)
            wu_x = sb.tile([128, 512], bf)
            nc.vector.memset(wu_w[:, :], 0.0)
            nc.vector.memset(wu_x[:, :], 0.0)
            wu_ps = ps.tile([128, 2, 512], fp32, tag="stripe", name="warm")
            for i in range(8):
                nc.tensor.matmul(wu_ps[:, 0, :], wu_w[:, :], wu_x[:, :],
                                 start=True, stop=True)

            # ---- phase 1: chunk-major over rb0-2 with 500-col stripes so
            # the PE starts as soon as the first 0.5MB W chunk lands. The
            # exp here skips the fused accumulator (it would pace ACT above
            # the warm PE); the idle DVE computes these row sums instead ----
            NW = 3
            for q in range(8):
                for rb in range(NW):
                    stripe(rb, q, 1, accum=False)
                if q == 3:
                    # copy path: cp[t, bb*C:(bb+1)*C] =
                    #   sum_s attnT[s, bb*64+t] * srcmap[s, bb, :]
                    # (attnT pre-scaled by p_copy on the host)
                    cpps = ps.tile([64, 4 * C], fp32, tag="stripe", name="cpps")
                    for bb in range(4):
                        for c in range(SCH):
                            nc.tensor.matmul(
                                cpps[:, bb * C:(bb + 1) * C],
                                attnT_sb[:, c, bb * 64:(bb + 1) * 64],
                                srcmap_sb[:, c, bb * C:(bb + 1) * C],
                                start=(c == 0), stop=(c == SCH - 1))
                    nc.vector.tensor_copy(cp_sb[:, :], cpps[:, :])
                    nc.gpsimd.dma_start(cp_d.ap(), cp_sb[:, :])
            for rb in range(NW):
                nc.vector.reduce_sum(rs_sb[:, rb * 8:rb * 8 + 1],
                                     ot_tiles[rb][:, :],
                                     axis=mybir.AxisListType.X)
                emit_out(rb)

            # ---- phase 2: row-major for the rest, 1000-col stripes; the
            # last row block streams its output in two halves so the final
            # DMA is small ----
            for rb in range(NW, NRB):
                for q in range(4):
                    stripe(rb, 2 * q, 2)
                    if rb == NRB - 1 and q >= 1:
                        # stream the last row block out in shrinking pieces
                        # so the final DMA is small
                        emit_out(rb, 2 * (q - 1) if q == 1 else 2 * q,
                                 2 * q + 2)
                if rb == NRB - 1:
                    # all rowsum slots except rb15's are final now
                    nc.gpsimd.dma_start(rs_d.ap()[:, 0:8 * (NRB - 1)],
                                        rs_sb[:, 0:8 * (NRB - 1)])
                else:
                    emit_out(rb)

            nc.sync.dma_start(rs_d.ap()[:, 8 * (NRB - 1):],
                              rs_sb[:, 8 * (NRB - 1):])

    nc.compile()
    return nc


def _get_nc(all_bias: bool):
    key = ("nc", all_bias)
    if key not in _cache:
        _cache[key] = _build(all_bias)
    return _cache[key]


def kernel(hidden, attn, src_map, W, b, Wc, bc):
    from concourse.bass_utils import run_bass_kernel_spmd

    hidden = np.asarray(hidden, dtype=np.float32)
    attn = np.asarray(attn, dtype=np.float32)
    src_map = np.asarray(src_map, dtype=np.float32)
    W = np.asarray(W, dtype=np.float32)
    b = np.asarray(b, dtype=np.float32)
    Wc = np.asarray(Wc, dtype=np.float32)
    bc = np.asarray(bc, dtype=np.float32)

    all_bias = bool(np.any(b != 0.0))

    # host prologue: p_copy (tiny matvec) and the per-row ACT bias ln(1-p)
    z = hidden.astype(np.float64) @ Wc.astype(np.float64) + bc.astype(np.float64)
    p = 1.0 / (1.0 + np.exp(-z))                         # [R, 1]
    one_m_p = (1.0 - p).reshape(-1)                      # [R]
    lnb = np.log(one_m_p).reshape(NRB, 128).T.astype(np.float32)  # [128, NRB]
    lnb = np.ascontiguousarray(lnb)

    hT = np.ascontiguousarray(hidden.T).astype(bf16)     # [512, 2048]
    attnS = attn * p.astype(np.float32)                  # [R, S] attn * p_copy

    nc = _get_nc(all_bias)

    in_maps = []
    for k in range(NC):
        Wk = np.ascontiguousarray(W[:, k * VS:(k + 1) * VS]).astype(bf16)

        # copy-path shard: batches 4k..4k+3, packed col j = bb*64 + t
        rows = np.array([[t * 32 + 4 * k + bb for t in range(T)] for bb in range(4)])
        rows_flat = rows.reshape(-1)
        attnT_k = np.ascontiguousarray(attnS[rows_flat, :].T).astype(bf16)   # [400, 256]
        srcmap_k = np.ascontiguousarray(
            src_map[:, 4 * k:4 * k + 4, :].reshape(S, 4 * C)).astype(bf16)  # [400, 400]

        im = {"hT": hT, "Wk": Wk, "lnb": lnb, "attnT": attnT_k, "srcmap": srcmap_k}
        if all_bias:
            bias_k = b[k * VS:(k + 1) * VS].astype(np.float64)
            if k == 0:
                bias_k = bias_k.copy()
                bias_k[PAD_IDX] += NEG_INF
            im["biask"] = bias_k.astype(bf16)[None, :]                      # [1, 4000]
        in_maps.append(im)

    global _last_in_maps
    _last_in_maps = in_maps
    res = run_bass_kernel_spmd(nc, in_maps, core_ids=list(range(NC))).results

    # host epilogue: finish the softmax denominator and normalize while
    # upcasting the bf16 shards.
    rs_tot = np.zeros((128, NRB), dtype=np.float64)
    for k in range(NC):
        rsk = res[k]["rs"].astype(np.float64).reshape(128, NRB, 8)
        rs_tot += rsk.sum(axis=2)
    zp = rs_tot.T.reshape(-1)                            # [R] = (1-p) * (Z + e_pad)

    full = np.empty((R, V + C), dtype=np.float32)
    for k in range(NC):
        full[:, k * VS:(k + 1) * VS] = res[k]["out"]

    if all_bias:
        # PAD handled via the -1e9 bias on the device (exp underflows to 0)
        zrow = zp / one_m_p                              # Z_true
    else:
        # device computed exp(0)=1 at the PAD column; remove it from Z
        e_pad = full[:, PAD_IDX].astype(np.float64) / one_m_p
        zrow = zp / one_m_p - e_pad
    scale = (1.0 / zrow).astype(np.float32)
    full[:, :V] *= scale[:, None]
    full[:, PAD_IDX] = 0.0

    t_idx = np.arange(T) * 32
    for k in range(NC):
        cp = res[k]["cp"].reshape(T, 4, C)
        for bb in range(4):
            full[t_idx + 4 * k + bb, V:] = cp[:, bb, :]
    return full


# revision 38
# speedup vs baseline: 1.0353x; 1.0353x over previous
"""CopyGenerator kernel for 8 Trainium2 NeuronCores.

Strategy: tensor-parallel over the vocab dimension, collective-free.
  - Each core computes logits = hidden @ W[:, k*4000:(k+1)*4000] (bf16 matmul,
    fp32 accumulate) and applies exp via ACT with a per-row bias ln(1-p_copy),
    so the activation directly emits e = exp(logit)*(1-p_copy) in bf16,
    streamed to DRAM, with the fused row-sum (accum_out) kept as fp32
    partials.
  - No AllReduce: the softmax denominator is finished on the host - each
    core returns its [128, 2, 16] row-sum partials (16 KB) and the host
    sums them across cores and applies the 1/Z row scale while upcasting
    the bf16 shards to the fp32 output.
  - p_copy = sigmoid(hidden @ Wc + bc) is a [2048,512]x[512,1] matvec,
    computed on the host; the device receives ln(1-p_copy) as an ACT bias
    and a pre-scaled attention (attn * p_copy) for the copy path.
  - Copy path (einsum over src_map) sharded 4 batches per core on the PE,
    emitted before the main loop so it runs while W streams in.
Host side: shard/cast inputs, run SPMD on cores 0-7, normalize + gather.
"""

import numpy as np
import ml_dtypes

bf16 = ml_dtypes.bfloat16

# Problem shape (hardcoded per contract)
B, T, S, C, D, V = 32, 64, 400, 100, 512, 32000
R = B * T              # 2048 rows, row r = t*32 + b
NC = 8
VS = V // NC           # 4000 vocab cols per core
PAD_IDX = 1
NEG_INF = -1e9

KCH = D // 128         # 4 contraction chunks of 128
NRB = R // 128         # 16 row blocks
SCH = 4                # s-chunks of 100 for the copy einsum
OUT_BUFS = 4
# hT row-chunks (DMA granules); each stored chunk-contiguous per partition
HT_CH = [(0, 384), (384, 512), (512, 1024), (1024, 1536), (1536, 2048)]
HT_OFF = [0]
for _r0, _r1 in HT_CH:
    HT_OFF.append(HT_OFF[-1] + KCH * (_r1 - _r0))

_cache = {}


def _build(all_bias: bool):
    import concourse.bass as bass
    import concourse.mybir as mybir
    import concourse.tile as tile
    from concourse import bacc

    fp32 = mybir.dt.float32
    bf = mybir.dt.bfloat16
    AF = mybir.ActivationFunctionType

    nc = bacc.Bacc("TRN2", target_bir_lowering=False, debug=False, num_devices=NC)

    # ---- I/O ----
    # hTp: packed [128, kch*rows] per row-chunk; Wp: packed [128, q, kch, 500]
    hT_d = nc.dram_tensor("hTp", [128, KCH * R], bf, kind="ExternalInput")
    W_d = nc.dram_tensor("Wp", [128, 8 * KCH * 500], bf, kind="ExternalInput")
    lnb_d = nc.dram_tensor("lnb", [128, NRB], fp32, kind="ExternalInput")
    attnT_d = nc.dram_tensor("attnT", [S, 256], bf, kind="ExternalInput")
    srcmap_d = nc.dram_tensor("srcmap", [S, 4 * C], bf, kind="ExternalInput")
    out_d = nc.dram_tensor("out", [R, VS], bf, kind="ExternalOutput")
    rs_d = nc.dram_tensor("rs", [128, 8 * NRB], fp32, kind="ExternalOutput")
    cp_d = nc.dram_tensor("cp", [T, 4 * C], fp32, kind="ExternalOutput")
    if all_bias:
        bias_d = nc.dram_tensor("biask", [1, VS], bf, kind="ExternalInput")

    with tile.TileContext(nc) as tc:
        with (
            tc.tile_pool(name="sb", bufs=1) as sb,
            tc.tile_pool(name="ps", bufs=4, space="PSUM") as ps,
        ):
            # ---- resident loads ----
            # sync (HWDGE) ring leads with the eight 0.5MB W column-chunks
            # that gate the matmul stream; the bulk of hT rides at its tail.
            # The gpsimd SWDGE ring carries, in parallel: lnb, the first hT
            # row-chunks (which gate the first stripes), and the copy-path
            # inputs.
            hT_sb = sb.tile([128, KCH * R], bf)
            hT_view = hT_d.ap()
            W_sb = sb.tile([128, 8, KCH, 500], bf)
            W_view = W_d.ap()

            def hT_op(rb, kk):
                """[128, 128] stationary operand for row block rb, k-chunk kk."""
                r = rb * 128
                for ci, (r0, r1) in enumerate(HT_CH):
                    if r0 <= r < r1:
                        base = HT_OFF[ci] + kk * (r1 - r0) + (r - r0)
                        return hT_sb[:, base:base + 128]
                raise AssertionError(rb)
            # sync ring: hT head + all W chunks; gpsimd ring (parallel):
            # lnb, hT row-chunk 1, copy-path inputs, hT bulk. All transfers
            # are fully contiguous on both sides (host packs the layouts).
            def hT_dma(eng, ci):
                nc_eng = getattr(nc, eng)
                o0, o1 = HT_OFF[ci], HT_OFF[ci + 1]
                nc_eng.dma_start(hT_sb[:, o0:o1], hT_view[:, o0:o1])

            hT_dma("sync", 0)
            for q in range(8):
                nc.sync.dma_start(W_sb[:, q, :, :],
                                  W_view[:, q * 2000:(q + 1) * 2000])

            lnb_sb = sb.tile([128, NRB], fp32)
            nc.gpsimd.dma_start(lnb_sb[:, :], lnb_d.ap())
            hT_dma("gpsimd", 1)
            attnT_sb = sb.tile([100, SCH, 256], bf)
            nc.gpsimd.dma_start(attnT_sb[:, :, :], attnT_d.ap().rearrange("(c p) j -> p c j", p=100))
            srcmap_sb = sb.tile([100, SCH, 4 * C], bf)
            nc.gpsimd.dma_start(srcmap_sb[:, :, :], srcmap_d.ap().rearrange("(c p) j -> p c j", p=100))
            for ci in range(2, 5):
                hT_dma("gpsimd", ci)
            if all_bias:
                bias_sb = sb.tile([1, VS], bf)
                nc.gpsimd.dma_start(bias_sb[:, :], bias_d.ap())
                ones_sb = sb.tile([1, 128], bf)
                nc.vector.memset(ones_sb[:, :], 1.0)

            rs_sb = sb.tile([128, 8 * NRB], fp32)  # rowsum partials [p, rb*8+c]
            nc.vector.memset(rs_sb[:, :], 0.0)
            cp_sb = sb.tile([64, 4 * C], fp32)

            ot_tiles = {}

            def get_ot(rb):
                if rb not in ot_tiles:
                    ot_tiles[rb] = sb.tile([128, VS], bf, tag="ot",
                                           bufs=OUT_BUFS, name=f"ot{rb}")
                return ot_tiles[rb]

            def stripe(rb, c0, nb, accum=True):
                """One nb*500-col stripe: matmuls + exp with fused bias/accum."""
                ot = get_ot(rb)
                st = ps.tile([128, 2, 512], fp32, tag="stripe",
                             name=f"l{rb}_{c0}")
                for kk in range(KCH):
                    for j in range(nb):
                        nc.tensor.matmul(
                            st[:, j, 0:500],
                            hT_op(rb, kk),
                            W_sb[:, c0 + j, kk, :],
                            start=(kk == 0),
                            stop=(kk == KCH - 1 and not all_bias))
                if all_bias:
                    for j in range(nb):
                        nc.tensor.matmul(
                            st[:, j, 0:500], ones_sb[:, :],
                            bias_sb[:, (c0 + j) * 500:(c0 + j + 1) * 500],
                            start=False, stop=True)
                ev = ot[:, c0 * 500:(c0 + nb) * 500]
                if nb > 1:
                    ev = ev.rearrange("p (g v) -> p g v", g=nb)
                    si = st[:, :, 0:500]
                else:
                    si = st[:, 0, 0:500]
                nc.scalar.activation(
                    ev, si, AF.Exp,
                    bias=lnb_sb[:, rb:rb + 1],
                    accum_out=(rs_sb[:, rb * 8 + c0:rb * 8 + c0 + 1]
                               if accum else None))

            def emit_out(rb, c0=0, c1=8):
                nc.sync.dma_start(
                    out_d.ap()[rb * 128:(rb + 1) * 128, c0 * 500:c1 * 500],
                    ot_tiles[rb][:, c0 * 500:c1 * 500])

            # ---- PE warmup: ~3.4us of dummy matmuls on zero tiles so the
            # HAM clock gate is at 8/8 (2.4 GHz) when the real stream
            # starts; they also fill the input-DMA wait ----
            wu_w = sb.tile([128, 128], bf)
            wu_x = sb.tile([128, 512], bf)
            nc.vector.memset(wu_w[:, :], 0.0)
            nc.vector.memset(wu_x[:, :], 0.0)
            wu_ps = ps.tile([128, 2, 512], fp32, tag="stripe", name="warm")
            for i in range(8):
                nc.tensor.matmul(wu_ps[:, 0, :], wu_w[:, :], wu_x[:, :],
                                 start=True, stop=True)

            # ---- phase 1: chunk-major over rb0-2 with 500-col stripes so
            # the PE starts as soon as the first 0.5MB W chunk lands. The
            # exp here skips the fused accumulator (it would pace ACT above
            # the warm PE); the idle DVE computes these row sums instead ----
            NW = 3
            for q in range(8):
                for rb in range(NW):
                    stripe(rb, q, 1, accum=False)
                if q == 3:
                    # copy path: cp[t, bb*C:(bb+1)*C] =
                    #   sum_s attnT[s, bb*64+t] * srcmap[s, bb, :]
                    # (attnT pre-scaled by p_copy on the host)
                    cpps = ps.tile([64, 4 * C], fp32, tag="stripe", name="cpps")
                    for bb in range(4):
                        for c in range(SCH):
                            nc.tensor.matmul(
                                cpps[:, bb * C:(bb + 1) * C],
                                attnT_sb[:, c, bb * 64:(bb + 1) * 64],
                                srcmap_sb[:, c, bb * C:(bb + 1) * C],
                                start=(c == 0), stop=(c == SCH - 1))
                    nc.vector.tensor_copy(cp_sb[:, :], cpps[:, :])
                    nc.gpsimd.dma_start(cp_d.ap(), cp_sb[:, :])
            for rb in range(NW):
                nc.vector.reduce_sum(rs_sb[:, rb * 8:rb * 8 + 1],
                                     ot_tiles[rb][:, :],
                                     axis=mybir.AxisListType.X)
                emit_out(rb)

            # ---- phase 2: row-major for the rest, 1000-col stripes; the
            # last row block streams its output in two halves so the final
            # DMA is small ----
            for rb in range(NW, NRB):
                for q in range(4):
                    stripe(rb, 2 * q, 2)
                    if rb == NRB - 1 and q >= 1:
                        # stream the last row block out in shrinking pieces
                        # so the final DMA is small
                        emit_out(rb, 2 * (q - 1) if q == 1 else 2 * q,
                                 2 * q + 2)
                if rb == NRB - 1:
                    # all rowsum slots except rb15's are final now
                    nc.gpsimd.dma_start(rs_d.ap()[:, 0:8 * (NRB - 1)],
                                        rs_sb[:, 0:8 * (NRB - 1)])
                else:
                    emit_out(rb)

            nc.sync.dma_start(rs_d.ap()[:, 8 * (NRB - 1):],
                              rs_sb[:, 8 * (NRB - 1):])

    nc.compile()
    return nc


def _get_nc(all_bias: bool):
    key = ("nc", all_bias)
    if key not in _cache:
        _cache[key] = _build(all_bias)
    return _cache[key]


def kernel(hidden, attn, src_map, W, b, Wc, bc):
    from concourse.bass_utils import run_bass_kernel_spmd

    hidden = np.asarray(hidden, dtype=np.float32)
    attn = np.asarray(attn, dtype=np.float32)
    src_map = np.asarray(src_map, dtype=np.float32)
    W = np.asarray(W, dtype=np.float32)
    b = np.asarray(b, dtype=np.float32)
    Wc = np.asarray(Wc, dtype=np.float32)
    bc = np.asarray(bc, dtype=np.float32)

    all_bias = bool(np.any(b != 0.0))

    # host prologue: p_copy (tiny matvec) and the per-row ACT bias ln(1-p)
    z = hidden.astype(np.float64) @ Wc.astype(np.float64) + bc.astype(np.float64)
    p = 1.0 / (1.0 + np.exp(-z))                         # [R, 1]
    one_m_p = (1.0 - p).reshape(-1)                      # [R]
    lnb = np.log(one_m_p).reshape(NRB, 128).T.astype(np.float32)  # [128, NRB]
    lnb = np.ascontiguousarray(lnb)

    # hT packed per row-chunk: [128, sum(kch*len)], chunk layout [p][kk][r]
    hT_bf = hidden.T.astype(bf16)                        # [512, 2048]
    parts = []
    for r0, r1 in HT_CH:
        parts.append(hT_bf[:, r0:r1].reshape(KCH, 128, r1 - r0)
                     .transpose(1, 0, 2).reshape(128, -1))
    hTp = np.ascontiguousarray(np.concatenate(parts, axis=1))  # [128, 8192]

    attnS = attn * p.astype(np.float32)                  # [R, S] attn * p_copy

    nc = _get_nc(all_bias)

    in_maps = []
    for k in range(NC):
        # W shard packed [p][q][kk][j]: chunk q = 500 vocab cols, contiguous
        Wk = (W[:, k * VS:(k + 1) * VS].astype(bf16)
              .reshape(KCH, 128, 8, 500).transpose(1, 2, 0, 3)
              .reshape(128, 8 * KCH * 500))
        Wk = np.ascontiguousarray(Wk)

        # copy-path shard: batches 4k..4k+3, packed col j = bb*64 + t
        rows = np.array([[t * 32 + 4 * k + bb for t in range(T)] for bb in range(4)])
        rows_flat = rows.reshape(-1)
        attnT_k = np.ascontiguousarray(attnS[rows_flat, :].T).astype(bf16)   # [400, 256]
        srcmap_k = np.ascontiguousarray(
            src_map[:, 4 * k:4 * k + 4, :].reshape(S, 4 * C)).astype(bf16)  # [400, 400]

        im = {"hTp": hTp, "Wp": Wk, "lnb": lnb, "attnT": attnT_k, "srcmap": srcmap_k}
        if all_bias:
            bias_k = b[k * VS:(k + 1) * VS].astype(np.float64)
            if k == 0:
                bias_k = bias_k.copy()
                bias_k[PAD_IDX] += NEG_INF
            im["biask"] = bias_k.astype(bf16)[None, :]                      # [1, 4000]
        in_maps.append(im)

    global _last_in_maps
    _last_in_maps = in_maps
    res = run_bass_kernel_spmd(nc, in_maps, core_ids=list(range(NC))).results

    # host epilogue: finish the softmax denominator and normalize while
    # upcasting the bf16 shards.
    rs_tot = np.zeros((128, NRB), dtype=np.float64)
    for k in range(NC):
        rsk = res[k]["rs"].astype(np.float64).reshape(128, NRB, 8)
        rs_tot += rsk.sum(axis=2)
    zp = rs_tot.T.reshape(-1)                            # [R] = (1-p) * (Z + e_pad)

    full = np.empty((R, V + C), dtype=np.float32)
    for k in range(NC):
        full[:, k * VS:(k + 1) * VS] = res[k]["out"]

    if all_bias:
        # PAD handled via the -1e9 bias on the device (exp underflows to 0)
        zrow = zp / one_m_p                              # Z_true
    else:
        # device computed exp(0)=1 at the PAD column; remove it from Z
        e_pad = full[:, PAD_IDX].astype(np.float64) / one_m_p
        zrow = zp / one_m_p - e_pad
    scale = (1.0 / zrow).astype(np.float32)
    full[:, :V] *= scale[:, None]
    full[:, PAD_IDX] = 0.0

    t_idx = np.arange(T) * 32
    for k in range(NC):
        cp = res[k]["cp"].reshape(T, 4, C)
        for bb in range(4):
            full[t_idx + 4 * k + bb, V:] = cp[:, bb, :]
    return full


# revision 39
# speedup vs baseline: 1.3627x; 1.3162x over previous
"""CopyGenerator kernel for 8 Trainium2 NeuronCores.

Strategy: tensor-parallel over the vocab dimension, collective-free,
fp8 (e4m3) DoubleRow matmul.
  - Each core computes logits = hidden @ W[:, k*4000:(k+1)*4000] as an fp8
    DoubleRow matmul (2 contraction rows per PE cell, ~1.4x bf16 rate,
    fp32 accumulate; host pre-scales h by 16 and W by 1024) and applies
    exp via ACT with scale 1/16384 and a per-row bias ln(1-p_copy), so the
    activation directly emits e = exp(logit)*(1-p_copy) in bf16, streamed
    to DRAM in 0.5MB half-row-block pieces.
  - Row sums run on the otherwise idle DVE (reduce over each out tile);
    no AllReduce: each core returns its [128, *] row-sum partials and the
    host finishes the softmax denominator and applies the 1/Z row scale
    while upcasting the bf16 shards to the fp32 output.
  - p_copy = sigmoid(hidden @ Wc + bc) is a [2048,512]x[512,1] matvec,
    computed on the host; the device receives ln(1-p_copy) as an ACT bias
    and a pre-scaled attention (attn * p_copy) for the copy path.
  - Copy path (einsum over src_map, bf16) sharded 4 batches per core,
    emitted inside phase 1 so it runs while W streams in.
Host side: shard/cast/pack inputs, run SPMD on cores 0-7, normalize +
gather.
"""

import numpy as np
import ml_dtypes

bf16 = ml_dtypes.bfloat16
f8 = ml_dtypes.float8_e4m3

# Problem shape (hardcoded per contract)
B, T, S, C, D, V = 32, 64, 400, 100, 512, 32000
R = B * T              # 2048 rows, row r = t*32 + b
NC = 8
VS = V // NC           # 4000 vocab cols per core
PAD_IDX = 1
NEG_INF = -1e9

KCH = D // 128         # 4 contraction chunks of 128
NRB = R // 128         # 16 row blocks
SCH = 4                # s-chunks of 100 for the copy einsum
OUT_BUFS = 4
SH = 16.0              # host pre-scale on hidden (fp8 range)
SW = 1024.0            # host pre-scale on W (fp8 range)
# hT row-chunks (DMA granules); each stored chunk-contiguous per partition
HT_CH = [(0, 384), (384, 512), (512, 1024), (1024, 1536), (1536, 2048)]
HT_OFF = [0]
for _r0, _r1 in HT_CH:
    HT_OFF.append(HT_OFF[-1] + KCH * (_r1 - _r0))

_cache = {}


def _build(all_bias: bool):
    import concourse.bass as bass
    import concourse.mybir as mybir
    import concourse.tile as tile
    from concourse import bacc

    fp32 = mybir.dt.float32
    bf = mybir.dt.bfloat16
    f8d = mybir.dt.float8e4
    AF = mybir.ActivationFunctionType
    DR = mybir.MatmulPerfMode.DoubleRow

    nc = bacc.Bacc("TRN2", target_bir_lowering=False, debug=False, num_devices=NC)

    # ---- I/O ----
    # hTp: packed [128, kch*rows] per row-chunk; Wp: packed [128, q, kch, 512]
    hT_d = nc.dram_tensor("hTp", [128, KCH * R], f8d, kind="ExternalInput")
    W_d = nc.dram_tensor("Wp", [128, 8 * KCH * 512], f8d, kind="ExternalInput")
    lnb_d = nc.dram_tensor("lnb", [128, NRB], fp32, kind="ExternalInput")
    attnT_d = nc.dram_tensor("attnT", [S, 256], bf, kind="ExternalInput")
    srcmap_d = nc.dram_tensor("srcmap", [S, 4 * C], bf, kind="ExternalInput")
    out_d = nc.dram_tensor("out", [R, VS], bf, kind="ExternalOutput")
    rs_d = nc.dram_tensor("rs", [128, NRB], fp32, kind="ExternalOutput")
    cp_d = nc.dram_tensor("cp", [T, 4 * C], fp32, kind="ExternalOutput")
    if all_bias:
        bias_d = nc.dram_tensor("biask", [1, VS], bf, kind="ExternalInput")

    with tile.TileContext(nc) as tc:
        with (
            tc.tile_pool(name="sb", bufs=1) as sb,
            tc.tile_pool(name="ps", bufs=2, space="PSUM") as ps,
        ):
            # ---- resident loads ----
            # sync ring: hT head + all W chunk-pairs; gpsimd ring (parallel):
            # lnb, hT row-chunk 1, copy-path inputs, hT bulk. All transfers
            # are fully contiguous on both sides (host packs the layouts).
            hT_ch = [sb.tile([128, KCH, r1 - r0], f8d, name=f"hT{ci}")
                     for ci, (r0, r1) in enumerate(HT_CH)]
            hT_view = hT_d.ap()
            W_sb = sb.tile([128, 8, KCH, 512], f8d)
            W_view = W_d.ap()

            def hT_dma(eng, ci):
                o0, o1 = HT_OFF[ci], HT_OFF[ci + 1]
                getattr(nc, eng).dma_start(hT_ch[ci][:, :, :], hT_view[:, o0:o1])

            def hT_op(rb, kp):
                """[128, 2, 128] DoubleRow stationary operand: row block rb,
                contraction rows kp*256..kp*256+255."""
                r = rb * 128
                for ci, (r0, r1) in enumerate(HT_CH):
                    if r0 <= r < r1:
                        return hT_ch[ci][:, 2 * kp:2 * kp + 2, r - r0:r - r0 + 128]
                raise AssertionError(rb)

            hT_dma("sync", 0)
            for q in range(8):
                nc.sync.dma_start(W_sb[:, q, :, :],
                                  W_view[:, q * 2048:(q + 1) * 2048])

            lnb_sb = sb.tile([128, NRB], fp32)
            nc.gpsimd.dma_start(lnb_sb[:, :], lnb_d.ap())
            hT_dma("gpsimd", 1)
            attnT_sb = sb.tile([100, SCH, 256], bf)
            nc.gpsimd.dma_start(attnT_sb[:, :, :], attnT_d.ap().rearrange("(c p) j -> p c j", p=100))
            srcmap_sb = sb.tile([100, SCH, 4 * C], bf)
            nc.gpsimd.dma_start(srcmap_sb[:, :, :], srcmap_d.ap().rearrange("(c p) j -> p c j", p=100))
            for ci in range(2, 5):
                hT_dma("gpsimd", ci)
            if all_bias:
                bias_sb = sb.tile([1, VS], bf)
                nc.gpsimd.dma_start(bias_sb[:, :], bias_d.ap())
                ones_sb = sb.tile([1, 128], bf)
                nc.vector.memset(ones_sb[:, :], 1.0)

            rs_sb = sb.tile([128, NRB], fp32)      # rowsum per row block (DVE)
            cp_sb = sb.tile([64, 4 * C], fp32)

            ot_tiles = {}

            def get_ot(rb):
                if rb not in ot_tiles:
                    ot_tiles[rb] = sb.tile([128, VS], bf, tag="ot",
                                           bufs=OUT_BUFS, name=f"ot{rb}")
                return ot_tiles[rb]

            def stripe(rb, c0, nb):
                """One nb*500-col stripe: DoubleRow matmuls + scaled exp."""
                ot = get_ot(rb)
                st = ps.tile([128, 4, 512], fp32, tag="stripe",
                             name=f"l{rb}_{c0}")
                for kp in range(2):
                    for j in range(nb):
                        nc.tensor.matmul(
                            st[:, j, 0:500],
                            hT_op(rb, kp),
                            W_sb[:, c0 + j, 2 * kp:2 * kp + 2, 0:500],
                            start=(kp == 0),
                            stop=(kp == 1 and not all_bias),
                            perf_mode=DR)
                if all_bias:
                    for j in range(nb):
                        nc.tensor.matmul(
                            st[:, j, 0:500], ones_sb[:, :],
                            bias_sb[:, (c0 + j) * 500:(c0 + j + 1) * 500],
                            start=False, stop=True)
                ev = ot[:, c0 * 500:(c0 + nb) * 500]
                si = st[:, 0:nb, 0:500]
                if nb > 1:
                    ev = ev.rearrange("p (g v) -> p g v", g=nb)
                else:
                    ev = ev
                    si = st[:, 0, 0:500]
                nc.scalar.activation(ev, si, AF.Exp, scale=1.0 / (SH * SW),
                                     bias=lnb_sb[:, rb:rb + 1])

            def emit_out(rb, c0=0, c1=8):
                nc.sync.dma_start(
                    out_d.ap()[rb * 128:(rb + 1) * 128, c0 * 500:c1 * 500],
                    ot_tiles[rb][:, c0 * 500:c1 * 500])

            def row_sum(rb):
                nc.vector.reduce_sum(rs_sb[:, rb:rb + 1], ot_tiles[rb][:, :],
                                     axis=mybir.AxisListType.X)

            # ---- PE warmup: ~3.4us of dummy matmuls on zero tiles so the
            # HAM clock gate is at 8/8 (2.4 GHz) when the real stream
            # starts; they also fill the input-DMA wait ----
            wu_w = sb.tile([128, 128], bf)
            wu_x = sb.tile([128, 512], bf)
            nc.vector.memset(wu_w[:, :], 0.0)
            nc.vector.memset(wu_x[:, :], 0.0)
            wu_ps = ps.tile([128, 4, 512], fp32, tag="stripe", name="warm")
            for i in range(8):
                nc.tensor.matmul(wu_ps[:, 0, :], wu_w[:, :], wu_x[:, :],
                                 start=True, stop=True)

            # ---- phase 1: chunk-pair-major over rb0-2 with 1000-col
            # stripes so the PE starts as soon as the first W pair lands ----
            NW = 3
            for qp in range(4):
                for rb in range(NW):
                    stripe(rb, 2 * qp, 2)
                if qp == 1:
                    # copy path: cp[t, bb*C:(bb+1)*C] =
                    #   sum_s attnT[s, bb*64+t] * srcmap[s, bb, :]
                    # (attnT pre-scaled by p_copy on the host)
                    cpps = ps.tile([64, 4 * C], fp32, tag="stripe", name="cpps")
                    for bb in range(4):
                        for c in range(SCH):
                            nc.tensor.matmul(
                                cpps[:, bb * C:(bb + 1) * C],
                                attnT_sb[:, c, bb * 64:(bb + 1) * 64],
                                srcmap_sb[:, c, bb * C:(bb + 1) * C],
                                start=(c == 0), stop=(c == SCH - 1))
                    nc.vector.tensor_copy(cp_sb[:, :], cpps[:, :])
                    nc.gpsimd.dma_start(cp_d.ap(), cp_sb[:, :])
            for rb in range(NW):
                row_sum(rb)
                emit_out(rb)

            # ---- phase 2: row-major, 2000-col stripes, output streamed in
            # half-row-block pieces ----
            for rb in range(NW, NRB):
                for h in range(2):
                    stripe(rb, 4 * h, 4)
                    emit_out(rb, 4 * h, 4 * h + 4)
                row_sum(rb)
                if rb == NRB - 1:
                    nc.gpsimd.dma_start(rs_d.ap()[:, 0:NRB - 1],
                                        rs_sb[:, 0:NRB - 1])

            nc.sync.dma_start(rs_d.ap()[:, NRB - 1:], rs_sb[:, NRB - 1:])

    nc.compile()
    return nc


def _get_nc(all_bias: bool):
    key = ("nc", all_bias)
    if key not in _cache:
        _cache[key] = _build(all_bias)
    return _cache[key]


def kernel(hidden, attn, src_map, W, b, Wc, bc):
    from concourse.bass_utils import run_bass_kernel_spmd

    hidden = np.asarray(hidden, dtype=np.float32)
    attn = np.asarray(attn, dtype=np.float32)
    src_map = np.asarray(src_map, dtype=np.float32)
    W = np.asarray(W, dtype=np.float32)
    b = np.asarray(b, dtype=np.float32)
    Wc = np.asarray(Wc, dtype=np.float32)
    bc = np.asarray(bc, dtype=np.float32)

    all_bias = bool(np.any(b != 0.0))

    # host prologue: p_copy (tiny matvec) and the per-row ACT bias ln(1-p)
    z = hidden.astype(np.float64) @ Wc.astype(np.float64) + bc.astype(np.float64)
    p = 1.0 / (1.0 + np.exp(-z))                         # [R, 1]
    one_m_p = (1.0 - p).reshape(-1)                      # [R]
    lnb = np.log(one_m_p).reshape(NRB, 128).T.astype(np.float32)  # [128, NRB]
    lnb = np.ascontiguousarray(lnb)

    # hT packed per row-chunk: [128, sum(kch*len)], chunk layout [p][kk][r]
    hT_f8 = np.clip(hidden.T * SH, -240.0, 240.0).astype(f8)  # [512, 2048]
    parts = []
    for r0, r1 in HT_CH:
        parts.append(hT_f8[:, r0:r1].reshape(KCH, 128, r1 - r0)
                     .transpose(1, 0, 2).reshape(128, -1))
    hTp = np.ascontiguousarray(np.concatenate(parts, axis=1))  # [128, 8192]

    attnS = attn * p.astype(np.float32)                  # [R, S] attn * p_copy

    nc = _get_nc(all_bias)

    in_maps = []
    for k in range(NC):
        # W shard packed [p][q][kk][j-pad-512]: chunk q = 500 vocab cols
        Wk8 = np.clip(W[:, k * VS:(k + 1) * VS] * SW, -240.0, 240.0).astype(f8)
        Wk8 = Wk8.reshape(KCH, 128, 8, 500).transpose(1, 2, 0, 3)  # [p,q,c,j]
        Wp = np.zeros((128, 8, KCH, 512), dtype=f8)
        Wp[:, :, :, 0:500] = Wk8
        Wp = Wp.reshape(128, 8 * KCH * 512)

        # copy-path shard: batches 4k..4k+3, packed col j = bb*64 + t
        rows = np.array([[t * 32 + 4 * k + bb for t in range(T)] for bb in range(4)])
        rows_flat = rows.reshape(-1)
        attnT_k = np.ascontiguousarray(attnS[rows_flat, :].T).astype(bf16)   # [400, 256]
        srcmap_k = np.ascontiguousarray(
            src_map[:, 4 * k:4 * k + 4, :].reshape(S, 4 * C)).astype(bf16)  # [400, 400]

        im = {"hTp": hTp, "Wp": Wp, "lnb": lnb, "attnT": attnT_k, "srcmap": srcmap_k}
        if all_bias:
            bias_k = b[k * VS:(k + 1) * VS].astype(np.float64)
            if k == 0:
                bias_k = bias_k.copy()
                bias_k[PAD_IDX] += NEG_INF
            im["biask"] = bias_k.astype(bf16)[None, :]                      # [1, 4000]
        in_maps.append(im)

    global _last_in_maps
    _last_in_maps = in_maps
    res = run_bass_kernel_spmd(nc, in_maps, core_ids=list(range(NC))).results

    # host epilogue: finish the softmax denominator and normalize while
    # upcasting the bf16 shards.
    rs_tot = np.zeros((128, NRB), dtype=np.float64)
    for k in range(NC):
        rs_tot += res[k]["rs"].astype(np.float64)
    zp = rs_tot.T.reshape(-1)                            # [R] = (1-p) * (Z + e_pad)

    full = np.empty((R, V + C), dtype=np.float32)
    for k in range(NC):
        full[:, k * VS:(k + 1) * VS] = res[k]["out"]

    if all_bias:
        # PAD handled via the -1e9 bias on the device (exp underflows to 0)
        zrow = zp / one_m_p                              # Z_true
    else:
        # device computed exp at the PAD column as well; remove it from Z
        e_pad = full[:, PAD_IDX].astype(np.float64) / one_m_p
        zrow = zp / one_m_p - e_pad
    scale = (1.0 / zrow).astype(np.float32)
    full[:, :V] *= scale[:, None]
    full[:, PAD_IDX] = 0.0

    t_idx = np.arange(T) * 32
    for k in range(NC):
        cp = res[k]["cp"].reshape(T, 4, C)
        for bb in range(4):
            full[t_idx + 4 * k + bb, V:] = cp[:, bb, :]
    return full


# revision 44
# speedup vs baseline: 1.5761x; 1.1566x over previous
"""CopyGenerator kernel for 8 Trainium2 NeuronCores.

Strategy: tensor-parallel over the vocab dimension, collective-free,
fp8 (e4m3) DoubleRow matmul.
  - Each core computes logits = hidden @ W[:, k*4000:(k+1)*4000] as an fp8
    DoubleRow matmul (2 contraction rows per PE cell, ~1.4x bf16 rate,
    fp32 accumulate; host pre-scales h by 16 and W by 1024) and applies
    exp via ACT with scale 1/16384 and a per-row bias ln(1-p_copy), so the
    activation directly emits e = exp(logit)*(1-p_copy) in bf16, streamed
    to DRAM in 0.5MB half-row-block pieces.
  - Row sums run on the otherwise idle DVE (reduce over each out tile);
    no AllReduce: each core returns its [128, *] row-sum partials and the
    host finishes the softmax denominator and applies the 1/Z row scale
    while upcasting the bf16 shards to the fp32 output.
  - p_copy = sigmoid(hidden @ Wc + bc) is a [2048,512]x[512,1] matvec,
    computed on the host; the device receives ln(1-p_copy) as an ACT bias
    and a pre-scaled attention (attn * p_copy) for the copy path.
  - Copy path (einsum over src_map, bf16) sharded 4 batches per core,
    emitted inside phase 1 so it runs while W streams in.
Host side: shard/cast/pack inputs, run SPMD on cores 0-7, normalize +
gather.
"""

import numpy as np
import ml_dtypes

bf16 = ml_dtypes.bfloat16
f8 = ml_dtypes.float8_e4m3

# Problem shape (hardcoded per contract)
B, T, S, C, D, V = 32, 64, 400, 100, 512, 32000
R = B * T              # 2048 rows, row r = t*32 + b
NC = 8
VS = V // NC           # 4000 vocab cols per core
PAD_IDX = 1
NEG_INF = -1e9

KCH = D // 128         # 4 contraction chunks of 128
NRB = R // 128         # 16 row blocks
SCH = 4                # s-chunks of 100 for the copy einsum
OUT_BUFS = 4
SH = 16.0              # host pre-scale on hidden (fp8 range)
SW = 1024.0            # host pre-scale on W (fp8 range)
# hT row-chunks (DMA granules); each stored chunk-contiguous per partition
HT_CH = [(0, 384), (384, 512), (512, 1024), (1024, 1536), (1536, 2048)]
HT_OFF = [0]
for _r0, _r1 in HT_CH:
    HT_OFF.append(HT_OFF[-1] + KCH * (_r1 - _r0))

_cache = {}


def _build(all_bias: bool):
    import concourse.bass as bass
    import concourse.mybir as mybir
    import concourse.tile as tile
    from concourse import bacc

    fp32 = mybir.dt.float32
    bf = mybir.dt.bfloat16
    f8d = mybir.dt.float8e4
    AF = mybir.ActivationFunctionType
    DR = mybir.MatmulPerfMode.DoubleRow

    nc = bacc.Bacc("TRN2", target_bir_lowering=False, debug=False, num_devices=NC)

    # ---- I/O ----
    # hTp: packed [128, kch*rows] per row-chunk; Wp: packed [128, q, kch, 512]
    hT_d = nc.dram_tensor("hTp", [128, KCH * R], f8d, kind="ExternalInput")
    W_d = nc.dram_tensor("Wp", [128, 8 * KCH * 512], f8d, kind="ExternalInput")
    lnb_d = nc.dram_tensor("lnb", [128, NRB], fp32, kind="ExternalInput")
    attnT_d = nc.dram_tensor("attnT", [S, 256], bf, kind="ExternalInput")
    srcmap_d = nc.dram_tensor("srcmap", [S, 4 * C], bf, kind="ExternalInput")
    out_d = nc.dram_tensor("out", [R, VS], bf, kind="ExternalOutput")
    cp_d = nc.dram_tensor("cp", [T, 4 * C], fp32, kind="ExternalOutput")
    if all_bias:
        bias_d = nc.dram_tensor("biask", [1, VS], bf, kind="ExternalInput")

    with tile.TileContext(nc) as tc:
        with (
            tc.tile_pool(name="sb", bufs=1) as sb,
            tc.tile_pool(name="ps", bufs=2, space="PSUM") as ps,
        ):
            # ---- resident loads ----
            # sync ring: hT head + all W chunk-pairs; gpsimd ring (parallel):
            # lnb, hT row-chunk 1, copy-path inputs, hT bulk. All transfers
            # are fully contiguous on both sides (host packs the layouts).
            hT_ch = [sb.tile([128, KCH, r1 - r0], f8d, name=f"hT{ci}")
                     for ci, (r0, r1) in enumerate(HT_CH)]
            hT_view = hT_d.ap()
            W_sb = sb.tile([128, 8, KCH, 512], f8d)
            W_view = W_d.ap()

            def hT_dma(eng, ci):
                o0, o1 = HT_OFF[ci], HT_OFF[ci + 1]
                getattr(nc, eng).dma_start(hT_ch[ci][:, :, :], hT_view[:, o0:o1])

            def hT_op(rb, kp):
                """[128, 2, 128] DoubleRow stationary operand: row block rb,
                contraction rows kp*256..kp*256+255."""
                r = rb * 128
                for ci, (r0, r1) in enumerate(HT_CH):
                    if r0 <= r < r1:
                        return hT_ch[ci][:, 2 * kp:2 * kp + 2, r - r0:r - r0 + 128]
                raise AssertionError(rb)

            hT_dma("sync", 0)
            for q in range(8):
                nc.sync.dma_start(W_sb[:, q, :, :],
                                  W_view[:, q * 2048:(q + 1) * 2048])

            lnb_sb = sb.tile([128, NRB], fp32)
            nc.gpsimd.dma_start(lnb_sb[:, :], lnb_d.ap())
            hT_dma("gpsimd", 1)
            attnT_sb = sb.tile([100, SCH, 256], bf)
            nc.gpsimd.dma_start(attnT_sb[:, :, :], attnT_d.ap().rearrange("(c p) j -> p c j", p=100))
            srcmap_sb = sb.tile([100, SCH, 4 * C], bf)
            nc.gpsimd.dma_start(srcmap_sb[:, :, :], srcmap_d.ap().rearrange("(c p) j -> p c j", p=100))
            for ci in range(2, 5):
                hT_dma("gpsimd", ci)
            if all_bias:
                bias_sb = sb.tile([1, VS], bf)
                nc.gpsimd.dma_start(bias_sb[:, :], bias_d.ap())
                ones_sb = sb.tile([1, 128], bf)
                nc.vector.memset(ones_sb[:, :], 1.0)

            cp_sb = sb.tile([64, 4 * C], fp32)

            ot_tiles = {}

            def get_ot(rb):
                if rb not in ot_tiles:
                    ot_tiles[rb] = sb.tile([128, VS], bf, tag="ot",
                                           bufs=OUT_BUFS, name=f"ot{rb}")
                return ot_tiles[rb]

            def stripe(rb, c0, nb):
                """One nb*500-col stripe: DoubleRow matmuls + scaled exp."""
                ot = get_ot(rb)
                st = ps.tile([128, 4, 512], fp32, tag="stripe",
                             name=f"l{rb}_{c0}")
                for kp in range(2):
                    for j in range(nb):
                        nc.tensor.matmul(
                            st[:, j, 0:500],
                            hT_op(rb, kp),
                            W_sb[:, c0 + j, 2 * kp:2 * kp + 2, 0:500],
                            start=(kp == 0),
                            stop=(kp == 1 and not all_bias),
                            perf_mode=DR)
                if all_bias:
                    for j in range(nb):
                        nc.tensor.matmul(
                            st[:, j, 0:500], ones_sb[:, :],
                            bias_sb[:, (c0 + j) * 500:(c0 + j + 1) * 500],
                            start=False, stop=True)
                ev = ot[:, c0 * 500:(c0 + nb) * 500]
                si = st[:, 0:nb, 0:500]
                if nb > 1:
                    ev = ev.rearrange("p (g v) -> p g v", g=nb)
                else:
                    ev = ev
                    si = st[:, 0, 0:500]
                nc.scalar.activation(ev, si, AF.Exp, scale=1.0 / (SH * SW),
                                     bias=lnb_sb[:, rb:rb + 1])

            def emit_out(rb, c0=0, c1=8):
                nc.sync.dma_start(
                    out_d.ap()[rb * 128:(rb + 1) * 128, c0 * 500:c1 * 500],
                    ot_tiles[rb][:, c0 * 500:c1 * 500])

            # ---- PE warmup: ~5us of dummy matmuls on zero tiles so the
            # HAM clock gate is at 8/8 (2.4 GHz) when the real stream
            # starts; they also fill the input-DMA wait ----
            wu_w = sb.tile([128, 128], bf)
            wu_x = sb.tile([128, 512], bf)
            nc.vector.memset(wu_w[:, :], 0.0)
            nc.vector.memset(wu_x[:, :], 0.0)
            wu_ps = ps.tile([128, 4, 512], fp32, tag="stripe", name="warm")
            for i in range(12):
                nc.tensor.matmul(wu_ps[:, 0, :], wu_w[:, :], wu_x[:, :],
                                 start=True, stop=True)

            # ---- phase 1: chunk-pair-major over rb0-2 with 1000-col
            # stripes so the PE starts as soon as the first W pair lands ----
            NW = 3
            for qp in range(4):
                for rb in range(NW):
                    stripe(rb, 2 * qp, 2)
                if qp == 1:
                    # copy path: cp[t, bb*C:(bb+1)*C] =
                    #   sum_s attnT[s, bb*64+t] * srcmap[s, bb, :]
                    # (attnT pre-scaled by p_copy on the host)
                    cpps = ps.tile([64, 4 * C], fp32, tag="stripe", name="cpps")
                    for bb in range(4):
                        for c in range(SCH):
                            nc.tensor.matmul(
                                cpps[:, bb * C:(bb + 1) * C],
                                attnT_sb[:, c, bb * 64:(bb + 1) * 64],
                                srcmap_sb[:, c, bb * C:(bb + 1) * C],
                                start=(c == 0), stop=(c == SCH - 1))
                    nc.vector.tensor_copy(cp_sb[:, :], cpps[:, :])
                    nc.gpsimd.dma_start(cp_d.ap(), cp_sb[:, :])
            for rb in range(NW):
                emit_out(rb)

            # ---- phase 2: row-major, 2000-col stripes, output streamed in
            # half-row-block pieces ----
            for rb in range(NW, NRB):
                for h in range(2):
                    stripe(rb, 4 * h, 4)
                    emit_out(rb, 4 * h, 4 * h + 4)

    nc.compile()
    return nc


def _get_nc(all_bias: bool):
    key = ("nc", all_bias)
    if key not in _cache:
        _cache[key] = _build(all_bias)
    return _cache[key]


def kernel(hidden, attn, src_map, W, b, Wc, bc):
    from concourse.bass_utils import run_bass_kernel_spmd

    hidden = np.asarray(hidden, dtype=np.float32)
    attn = np.asarray(attn, dtype=np.float32)
    src_map = np.asarray(src_map, dtype=np.float32)
    W = np.asarray(W, dtype=np.float32)
    b = np.asarray(b, dtype=np.float32)
    Wc = np.asarray(Wc, dtype=np.float32)
    bc = np.asarray(bc, dtype=np.float32)

    all_bias = bool(np.any(b != 0.0))

    # host prologue: p_copy (tiny matvec) and the per-row ACT bias ln(1-p)
    z = hidden.astype(np.float64) @ Wc.astype(np.float64) + bc.astype(np.float64)
    p = 1.0 / (1.0 + np.exp(-z))                         # [R, 1]
    one_m_p = (1.0 - p).reshape(-1)                      # [R]
    lnb = np.log(one_m_p).reshape(NRB, 128).T.astype(np.float32)  # [128, NRB]
    lnb = np.ascontiguousarray(lnb)

    # hT packed per row-chunk: [128, sum(kch*len)], chunk layout [p][kk][r]
    hT_f8 = np.clip(hidden.T * SH, -240.0, 240.0).astype(f8)  # [512, 2048]
    parts = []
    for r0, r1 in HT_CH:
        parts.append(hT_f8[:, r0:r1].reshape(KCH, 128, r1 - r0)
                     .transpose(1, 0, 2).reshape(128, -1))
    hTp = np.ascontiguousarray(np.concatenate(parts, axis=1))  # [128, 8192]

    attnS = attn * p.astype(np.float32)                  # [R, S] attn * p_copy

    nc = _get_nc(all_bias)

    in_maps = []
    for k in range(NC):
        # W shard packed [p][q][kk][j-pad-512]: chunk q = 500 vocab cols
        Wk8 = np.clip(W[:, k * VS:(k + 1) * VS] * SW, -240.0, 240.0).astype(f8)
        Wk8 = Wk8.reshape(KCH, 128, 8, 500).transpose(1, 2, 0, 3)  # [p,q,c,j]
        Wp = np.zeros((128, 8, KCH, 512), dtype=f8)
        Wp[:, :, :, 0:500] = Wk8
        Wp = Wp.reshape(128, 8 * KCH * 512)

        # copy-path shard: batches 4k..4k+3, packed col j = bb*64 + t
        rows = np.array([[t * 32 + 4 * k + bb for t in range(T)] for bb in range(4)])
        rows_flat = rows.reshape(-1)
        attnT_k = np.ascontiguousarray(attnS[rows_flat, :].T).astype(bf16)   # [400, 256]
        srcmap_k = np.ascontiguousarray(
            src_map[:, 4 * k:4 * k + 4, :].reshape(S, 4 * C)).astype(bf16)  # [400, 400]

        im = {"hTp": hTp, "Wp": Wp, "lnb": lnb, "attnT": attnT_k, "srcmap": srcmap_k}
        if all_bias:
            bias_k = b[k * VS:(k + 1) * VS].astype(np.float64)
            if k == 0:
                bias_k = bias_k.copy()
                bias_k[PAD_IDX] += NEG_INF
            im["biask"] = bias_k.astype(bf16)[None, :]                      # [1, 4000]
        in_maps.append(im)

    global _last_in_maps
    _last_in_maps = in_maps
    res = run_bass_kernel_spmd(nc, in_maps, core_ids=list(range(NC))).results

    # host epilogue: finish the softmax denominator from the gathered
    # shards (device out = e*(1-p); summing the bf16-rounded values moves
    # Z by ~1e-5 relative) and normalize while upcasting.
    full = np.empty((R, V + C), dtype=np.float32)
    for k in range(NC):
        full[:, k * VS:(k + 1) * VS] = res[k]["out"]

    s_row = full[:, :V].sum(axis=1, dtype=np.float64)    # (1-p) * (Z + e_pad)
    # remove the PAD column's contribution (device computed exp there too;
    # it is 0 in the all_bias build, so the same formula covers both)
    scale = (one_m_p / (s_row - full[:, PAD_IDX])).astype(np.float32)
    full[:, :V] *= scale[:, None]
    full[:, PAD_IDX] = 0.0

    t_idx = np.arange(T) * 32
    for k in range(NC):
        cp = res[k]["cp"].reshape(T, 4, C)
        for bb in range(4):
            full[t_idx + 4 * k + bb, V:] = cp[:, bb, :]
    return full
